# revision 4
# baseline (speedup 1.0000x reference)
"""GCN (2x GCNConv + mean-pool + linear) on 8 Trainium2 cores.

Strategy (all on-device in ONE SPMD dispatch):
  - Nodes block-partitioned across 8 cores (6250/core, padded to 6272=49*128).
  - Per layer: each core computes ys = dinv * (h_shard @ W) for its shard
    (49 PE matmuls), AllGather replicates the 50176-row feature table to
    every core's HBM.
  - Message passing = degree-bucketed dma_gather rounds: round j gathers the
    j-th in-edge's source row for every local dst node (nodes rank-ordered by
    in-degree so each round's active set is a prefix; tail slots gather a
    zero pad row).  Accumulation on DVE.  dma_gather indices are int16, so
    the table is split in two 25088-row halves with an independent
    rank/round system per half; the second system's accumulator is merged in
    via one permutation dma_gather through HBM.
  - Epilogue per 128-node chunk: scale by dinv[dst], +bias, ReLU (and a PE
    transpose to feature-major between layer 1 and layer 2).
  - Mean-pool: one-hot graph matrix per chunk (DVE is_equal vs iota) and PE
    matmul accumulation -> per-core [128 feat, 256 graph] partial sums.
  - Host: sum the 8 partials, divide by counts, final [256,128]@[128,10].
"""

import numpy as np

N_NODES = 50000
N_EDGES = 800000
D = 128
N_GRAPHS = 256
CORES = 8
SH_REAL = 6250
KCH = 49
SH = KCH * 128            # 6272
TBL = CORES * SH          # 50176
HALF = 4 * SH             # 25088
PADG = 300.0
KB = 7                    # gemm-out DMA batch
MAXC = 8                  # max gather piece: MAXC*128 indices                    # gemm-out DMA batching (49 = 7*7)


def _srow(i):
    """node/slot index (k*128+p) -> flat row (p*KCH + k) of the (p,k,f) table."""
    return (i % 128) * KCH + i // 128


def _wrap_idx(idx):
    n = len(idx)
    assert n % 16 == 0
    base = np.asarray(idx, np.int16).reshape(-1, 16).T
    return np.ascontiguousarray(np.tile(base, (8, 1)))


def _host_prep(x, edge_index, batch):
    x = np.asarray(x, np.float32)
    src = np.concatenate([edge_index[0], np.arange(N_NODES, dtype=np.int64)]).astype(np.int64)
    dst = np.concatenate([edge_index[1], np.arange(N_NODES, dtype=np.int64)]).astype(np.int64)
    deg = np.bincount(dst, minlength=N_NODES).astype(np.float32)
    dinv = (1.0 / np.sqrt(deg)).astype(np.float32)

    core_of = dst // SH_REAL
    loc_of_dst = dst % SH_REAL
    src_core = src // SH_REAL
    src_loc = src % SH_REAL
    t1row_src = src_core * SH + _srow(src_loc)
    half_of_src = (t1row_src >= HALF).astype(np.int64)

    meta = []
    for c in range(CORES):
        m = {}
        emask = core_of == c
        for s in (0, 1):
            es = emask & (half_of_src == s)
            ldst = loc_of_dst[es]
            lsrc_row = t1row_src[es]
            gsrc = src[es]
            degs = np.bincount(ldst, minlength=SH).astype(np.int64)
            order = np.argsort(-degs, kind="stable")
            rank_of = np.empty(SH, np.int64)
            rank_of[order] = np.arange(SH)
            o2 = np.argsort(ldst, kind="stable")
            m[s] = dict(degs=degs, node_of_rank=order, rank_of=rank_of,
                        lsrc=lsrc_row[o2], gsrc=gsrc[o2],
                        starts=np.searchsorted(ldst[o2], np.arange(SH)))
        meta.append(m)

    R = [max(1, max(int(meta[c][s]["degs"].max()) for c in range(CORES))) for s in (0, 1)]
    COLS = [[], []]
    for s in (0, 1):
        for j in range(R[s]):
            cj = max(int((meta[c][s]["degs"] > j).sum()) for c in range(CORES))
            COLS[s].append(KCH if j == 0 else max(1, -(-cj // 128)))

    # layer-2 table row of global node n (canonical = system-0 rank order)
    t2row = np.empty(N_NODES, np.int64)
    for c in range(CORES):
        rk = meta[c][0]["rank_of"]
        t2row[c * SH_REAL:(c + 1) * SH_REAL] = c * SH + _srow(rk[:SH_REAL])

    ZROW = _srow(np.int64(SH_REAL + 2))  # a zero pad row within each half's first core

    core_inputs = []
    for c in range(CORES):
        m = meta[c]
        xT = np.zeros((D, SH), np.float32)
        xT[:, :SH_REAL] = x[c * SH_REAL:(c + 1) * SH_REAL].T
        dv = np.zeros(SH, np.float32)
        dv[:SH_REAL] = dinv[c * SH_REAL:(c + 1) * SH_REAL]
        dinv1 = np.ascontiguousarray(dv.reshape(KCH, 128).T)

        nor0 = m[0]["node_of_rank"]
        valid0 = nor0 < SH_REAL
        gnode0 = c * SH_REAL + np.minimum(nor0, SH_REAL - 1)
        dvR = np.where(valid0, dinv[gnode0], 0.0).astype(np.float32)
        dinvR = np.ascontiguousarray(dvR.reshape(KCH, 128).T)
        br = np.where(valid0, np.asarray(batch)[gnode0].astype(np.float64), PADG)
        batchR = np.ascontiguousarray(br.reshape(KCH, 128).T).astype(np.float32)

        idx_l = {1: [[], []], 2: [[], []]}
        for s in (0, 1):
            ms = m[s]
            base = HALF * s
            nedge = len(ms["lsrc"])
            for j in range(R[s]):
                nslots = COLS[s][j] * 128
                ranks = np.arange(nslots)
                live = ranks < int((ms["degs"] > j).sum())
                ln = ms["node_of_rank"][np.minimum(ranks, SH - 1)]
                epos = np.minimum(ms["starts"][ln] + j, max(nedge - 1, 0))
                if nedge:
                    r1 = np.where(live, ms["lsrc"][epos] - base, ZROW)
                    r2 = np.where(live, t2row[ms["gsrc"][epos]] - base, ZROW)
                else:
                    r1 = np.full(nslots, ZROW, np.int64)
                    r2 = np.full(nslots, ZROW, np.int64)
                idx_l[1][s].append(r1)
                idx_l[2][s].append(r2)

        perm01 = _srow(m[1]["rank_of"][nor0])

        core_inputs.append(dict(
            xT=xT, dinv1=dinv1, dinvR=dinvR, batchR=batchR,
            idx1_0=_wrap_idx(np.concatenate(idx_l[1][0])),
            idx1_1=_wrap_idx(np.concatenate(idx_l[1][1])),
            idx2_0=_wrap_idx(np.concatenate(idx_l[2][0])),
            idx2_1=_wrap_idx(np.concatenate(idx_l[2][1])),
            perm=_wrap_idx(perm01),
        ))

    cnts = np.bincount(np.asarray(batch, np.int64), minlength=N_GRAPHS).astype(np.float32)
    return core_inputs, R, COLS, cnts


def _build(R, COLS):
    import concourse.mybir as mybir
    import concourse.tile as tile
    from concourse import bacc, library_config

    f32 = mybir.dt.float32
    i16 = mybir.dt.int16
    Alu = mybir.AluOpType
    W0_, W1_ = sum(COLS[0]) * 8, sum(COLS[1]) * 8   # wrapped idx widths per layer

    nc = bacc.Bacc(None, target_bir_lowering=False, num_devices=CORES)
    with tile.TileContext(nc) as tc:
        with tc.tile_pool(name="dram", bufs=1, space="DRAM") as dram, \
             tc.tile_pool(name="cst", bufs=1) as cst, \
             tc.tile_pool(name="big", bufs=1) as big, \
             tc.tile_pool(name="tmp", bufs=2) as tmpp, \
             tc.tile_pool(name="stg", bufs=3) as stg, \
             tc.tile_pool(name="pg", bufs=2, space="PSUM") as pg, \
             tc.tile_pool(name="pt", bufs=2, space="PSUM") as pt, \
             tc.tile_pool(name="pp", bufs=1, space="PSUM") as pp:

            # ---------- I/O ----------
            xT_d = dram.tile((D, SH), f32, kind="ExternalInput")
            W1_d = dram.tile((D, D), f32, kind="ExternalInput")
            W2_d = dram.tile((D, D), f32, kind="ExternalInput")
            b1_d = dram.tile((D, 1), f32, kind="ExternalInput")
            b2b_d = dram.tile((D, D), f32, kind="ExternalInput")
            dinv1_d = dram.tile((D, KCH), f32, kind="ExternalInput")
            dinvR_d = dram.tile((D, KCH), f32, kind="ExternalInput")
            batchR_d = dram.tile((D, KCH), f32, kind="ExternalInput")
            iota_d = dram.tile((D, N_GRAPHS), f32, kind="ExternalInput")
            ident_d = dram.tile((D, D), f32, kind="ExternalInput")
            idx_d = {
                (1, 0): dram.tile((128, W0_), i16, kind="ExternalInput", name="idx10"),
                (1, 1): dram.tile((128, W1_), i16, kind="ExternalInput", name="idx11"),
                (2, 0): dram.tile((128, W0_), i16, kind="ExternalInput", name="idx20"),
                (2, 1): dram.tile((128, W1_), i16, kind="ExternalInput", name="idx21"),
            }
            perm_d = dram.tile((128, SH // 16), i16, kind="ExternalInput")
            pool_d = dram.tile((D, N_GRAPHS), f32, kind="ExternalOutput")

            # internal DRAM
            bounce = [dram.tile((128, KCH, D), f32, name=f"bounce{i}") for i in range(2)]
            table = [dram.tile((TBL, D), f32, name=f"table{i}") for i in range(2)]
            acc1_d = dram.tile((SH, D), f32)

            # ---------- SBUF ----------
            nc.gpsimd.load_library(library_config.mlp)
            xT = big.tile([D, SH], f32)
            h1T = big.tile([D, SH], f32)
            acc0 = big.tile([128, KCH, D], f32)
            acc1 = big.tile([128, KCH, D], f32)
            W1s = cst.tile([D, D], f32)
            W2s = cst.tile([D, D], f32)
            b1s = cst.tile([D, 1], f32)
            b2b = cst.tile([D, D], f32)
            dinv1 = cst.tile([D, KCH], f32)
            dinvR = cst.tile([D, KCH], f32)
            batchR = cst.tile([D, KCH], f32)
            iota = cst.tile([D, N_GRAPHS], f32)
            ident = cst.tile([D, D], f32)
            idx_sb = {
                (1, 0): cst.tile([128, W0_], i16, name="idx10_sb"),
                (1, 1): cst.tile([128, W1_], i16, name="idx11_sb"),
                (2, 0): cst.tile([128, W0_], i16, name="idx20_sb"),
                (2, 1): cst.tile([128, W1_], i16, name="idx21_sb"),
            }
            perm_sb = cst.tile([128, SH // 16], i16)

            for sb, d in [(xT, xT_d), (W1s, W1_d), (W2s, W2_d), (b1s, b1_d),
                          (b2b, b2b_d), (dinv1, dinv1_d), (dinvR, dinvR_d),
                          (batchR, batchR_d), (iota, iota_d), (ident, ident_d),
                          (perm_sb, perm_d)] + [(idx_sb[k], idx_d[k]) for k in idx_sb]:
                nc.sync.dma_start(out=sb[:], in_=d[:])

            def gemm_ag(lhsT_sb, Wsb, dv_sb, lnum):
                """49 matmuls + dinv scale + batched DMA to bounce, then AllGather."""
                for k0 in range(0, KCH, KB):
                    stage = stg.tile([128, KB, D], f32, tag="stage")
                    for k in range(k0, k0 + KB):
                        ps = pg.tile([128, D], f32, space="PSUM", tag="pg")
                        nc.tensor.matmul(ps[:], lhsT=lhsT_sb[:, k * 128:(k + 1) * 128],
                                         rhs=Wsb[:], start=True, stop=True)
                        nc.vector.tensor_scalar(
                            out=stage[:, k - k0, :], in0=ps[:],
                            scalar1=dv_sb[:, k:k + 1], scalar2=None, op0=Alu.mult)
                    nc.scalar.dma_start(out=bounce[lnum][:, k0:k0 + KB, :],
                                        in_=stage[:])
                nc.gpsimd.collective_compute(
                    "AllGather", Alu.bypass,
                    replica_groups=[list(range(CORES))],
                    ins=[bounce[lnum].opt()],
                    outs=[table[lnum].opt()])

            def gather_pieces(dst_tile, dst_c0, src, idxap, icol0, cols, accumulate):
                """issue dma_gather in pieces of <= MAXC columns (128 idx each)."""
                for p0 in range(0, cols, MAXC):
                    cc = min(MAXC, cols - p0)
                    n = cc * 128
                    isl = idxap[:, icol0 + p0 * 8: icol0 + (p0 + cc) * 8]
                    if not accumulate:
                        nc.gpsimd.dma_gather(
                            dst_tile[:, dst_c0 + p0: dst_c0 + p0 + cc, :],
                            src, isl, n, n, D)
                    else:
                        t = tmpp.tile([128, MAXC, D], f32, tag="tmp")
                        nc.gpsimd.dma_gather(t[:, :cc, :], src, isl, n, n, D)
                        nc.vector.tensor_tensor(
                            out=dst_tile[:, dst_c0 + p0: dst_c0 + p0 + cc, :],
                            in0=dst_tile[:, dst_c0 + p0: dst_c0 + p0 + cc, :],
                            in1=t[:, :cc, :], op=Alu.add)

            def rounds(lnum):
                """degree-round gathers for both half-systems + merge into acc0."""
                halves = (table[lnum][0:HALF, :], table[lnum][HALF:TBL, :])
                for s, accs in ((0, acc0), (1, acc1)):
                    icol = 0
                    for j in range(R[s]):
                        cols = COLS[s][j]
                        gather_pieces(accs, 0, halves[s], idx_sb[(lnum + 1, s)],
                                      icol, cols, accumulate=(j > 0))
                        icol += cols * 8
                # merge system 1 into canonical (system-0) slot order
                nc.scalar.dma_start(out=acc1_d[:], in_=acc1[:])
                tm = tmpp.tile([128, KCH, D], f32, tag="tmpm")
                gather_pieces(tm, 0, acc1_d[:], perm_sb, 0, KCH, accumulate=False)
                nc.vector.tensor_tensor(out=acc0[:], in0=acc0[:], in1=tm[:],
                                        op=Alu.add)

            # ---------------- layer 1 ----------------
            gemm_ag(xT, W1s, dinv1, 0)
            rounds(0)
            # epilogue: h1T[:, chunk] = relu(transpose(acc0*dinvR) + b1)
            for k in range(KCH):
                st = stg.tile([128, D], f32, tag="epi")
                nc.vector.tensor_scalar(out=st[:], in0=acc0[:, k, :],
                                        scalar1=dinvR[:, k:k + 1], scalar2=None,
                                        op0=Alu.mult)
                pst = pt.tile([128, D], f32, space="PSUM", tag="pt")
                nc.tensor.transpose(pst[:], st[:], ident[:])
                nc.scalar.activation(out=h1T[:, k * 128:(k + 1) * 128], in_=pst[:],
                                     func=mybir.ActivationFunctionType.Relu,
                                     bias=b1s[:, :1], scale=1.0)

            # ---------------- layer 2 ----------------
            gemm_ag(h1T, W2s, dinvR, 1)
            rounds(1)
            pool_ps = pp.tile([128, N_GRAPHS], f32, space="PSUM")
            for k in range(KCH):
                h2 = stg.tile([128, D], f32, tag="h2")
                nc.vector.tensor_scalar(out=h2[:], in0=acc0[:, k, :],
                                        scalar1=dinvR[:, k:k + 1], scalar2=None,
                                        op0=Alu.mult)
                nc.vector.tensor_tensor(out=h2[:], in0=h2[:], in1=b2b[:], op=Alu.add)
                h2r = stg.tile([128, D], f32, tag="h2r")
                nc.scalar.activation(out=h2r[:], in_=h2[:],
                                     func=mybir.ActivationFunctionType.Relu,
                                     bias=0.0, scale=1.0)
                G = stg.tile([128, N_GRAPHS], f32, tag="G")
                nc.vector.tensor_scalar(out=G[:], in0=iota[:],
                                        scalar1=batchR[:, k:k + 1], scalar2=None,
                                        op0=Alu.is_equal)
                nc.tensor.matmul(pool_ps[:], lhsT=h2r[:], rhs=G[:],
                                 start=(k == 0), stop=(k == KCH - 1))
            outsb = stg.tile([128, N_GRAPHS], f32, tag="G")
            nc.vector.tensor_copy(out=outsb[:], in_=pool_ps[:])
            nc.sync.dma_start(out=pool_d[:], in_=outsb[:])

    nc.compile()
    names = dict(
        xT=xT_d.name, W1=W1_d.name, W2=W2_d.name, b1=b1_d.name, b2b=b2b_d.name,
        dinv1=dinv1_d.name, dinvR=dinvR_d.name, batchR=batchR_d.name,
        iota=iota_d.name, ident=ident_d.name, perm=perm_d.name,
        i10=idx_d[(1, 0)].name, i11=idx_d[(1, 1)].name,
        i20=idx_d[(2, 0)].name, i21=idx_d[(2, 1)].name,
        out=pool_d.name)
    return nc, names


_cache = {}
_last_in_maps = None


def kernel(x, edge_index, batch, W1, b1, W2, b2, Wl, bl):
    from concourse.bass_utils import run_bass_kernel_spmd

    x = np.asarray(x, np.float32)
    edge_index = np.asarray(edge_index)
    batch = np.asarray(batch)
    W1 = np.asarray(W1, np.float32); b1 = np.asarray(b1, np.float32)
    W2 = np.asarray(W2, np.float32); b2 = np.asarray(b2, np.float32)
    Wl = np.asarray(Wl, np.float32); bl = np.asarray(bl, np.float32)

    core_inputs, R, COLS, cnts = _host_prep(x, edge_index, batch)

    key = (tuple(R), tuple(map(tuple, COLS)))
    if key not in _cache:
        _cache[key] = _build(R, COLS)
    nc, nm = _cache[key]

    iota_b = np.broadcast_to(np.arange(N_GRAPHS, dtype=np.float32)[None, :],
                             (D, N_GRAPHS)).copy()
    b2b = np.broadcast_to(b2[None, :], (D, D)).copy()
    ident = np.eye(D, dtype=np.float32)
    in_maps = []
    for ci in core_inputs:
        in_maps.append({
            nm["xT"]: ci["xT"], nm["W1"]: W1, nm["W2"]: W2,
            nm["b1"]: b1.reshape(D, 1), nm["b2b"]: b2b,
            nm["dinv1"]: ci["dinv1"], nm["dinvR"]: ci["dinvR"],
            nm["batchR"]: ci["batchR"], nm["iota"]: iota_b, nm["ident"]: ident,
            nm["perm"]: ci["perm"],
            nm["i10"]: ci["idx1_0"], nm["i11"]: ci["idx1_1"],
            nm["i20"]: ci["idx2_0"], nm["i21"]: ci["idx2_1"],
        })
    global _last_in_maps
    _last_in_maps = in_maps
    res = run_bass_kernel_spmd(nc, in_maps, list(range(CORES)))
    total = np.zeros((D, N_GRAPHS), np.float32)
    for r in res.results:
        total += r[nm["out"]]
    pooled = (total / np.maximum(cnts, 1.0)[None, :]).T
    return (pooled @ Wl + bl).astype(np.float32)


# revision 5
# speedup vs baseline: 1.2126x; 1.2126x over previous
"""GCN (2x GCNConv + mean-pool + linear) on 8 Trainium2 cores.

Strategy (all on-device in ONE SPMD dispatch):
  - Nodes block-partitioned across 8 cores (6250/core, padded to 6272=49*128).
  - Per layer: each core computes ys = dinv * (h_shard @ W) for its shard
    (49 PE matmuls), AllGather replicates the 50176-row feature table to
    every core's HBM.
  - Message passing = degree-bucketed dma_gather rounds: round j gathers the
    j-th in-edge's source row for every local dst node (nodes rank-ordered by
    in-degree so each round's active set is a prefix; tail slots gather a
    zero pad row).  Accumulation on DVE.  dma_gather indices are int16, so
    the table is split in two 25088-row halves with an independent
    rank/round system per half; the second system's accumulator is merged in
    via one permutation dma_gather through HBM.
  - Epilogue per 128-node chunk: scale by dinv[dst], +bias, ReLU (and a PE
    transpose to feature-major between layer 1 and layer 2).
  - Mean-pool: one-hot graph matrix per chunk (DVE is_equal vs iota) and PE
    matmul accumulation -> per-core [128 feat, 256 graph] partial sums.
  - Host: sum the 8 partials, divide by counts, final [256,128]@[128,10].
"""

import numpy as np

N_NODES = 50000
N_EDGES = 800000
D = 128
N_GRAPHS = 256
CORES = 8
SH_REAL = 6250
KCH = 49
SH = KCH * 128            # 6272
TBL = CORES * SH          # 50176
HALF = 4 * SH             # 25088
PADG = 300.0
KB = 7                    # gemm-out DMA batch
MAXC = 8                  # max gather piece: MAXC*128 indices                    # gemm-out DMA batching (49 = 7*7)


def _srow(i):
    """node/slot index (k*128+p) -> flat row (p*KCH + k) of the (p,k,f) table."""
    return (i % 128) * KCH + i // 128


def _wrap_idx(idx):
    n = len(idx)
    assert n % 16 == 0
    base = np.asarray(idx, np.int16).reshape(-1, 16).T
    return np.ascontiguousarray(np.tile(base, (8, 1)))


def _host_prep(x, edge_index, batch):
    x = np.asarray(x, np.float32)
    src = np.concatenate([edge_index[0], np.arange(N_NODES, dtype=np.int64)]).astype(np.int64)
    dst = np.concatenate([edge_index[1], np.arange(N_NODES, dtype=np.int64)]).astype(np.int64)
    deg = np.bincount(dst, minlength=N_NODES).astype(np.float32)
    dinv = (1.0 / np.sqrt(deg)).astype(np.float32)

    core_of = dst // SH_REAL
    loc_of_dst = dst % SH_REAL
    src_core = src // SH_REAL
    src_loc = src % SH_REAL
    t1row_src = src_core * SH + _srow(src_loc)
    half_of_src = (t1row_src >= HALF).astype(np.int64)

    meta = []
    for c in range(CORES):
        m = {}
        emask = core_of == c
        for s in (0, 1):
            es = emask & (half_of_src == s)
            ldst = loc_of_dst[es]
            lsrc_row = t1row_src[es]
            gsrc = src[es]
            degs = np.bincount(ldst, minlength=SH).astype(np.int64)
            order = np.argsort(-degs, kind="stable")
            rank_of = np.empty(SH, np.int64)
            rank_of[order] = np.arange(SH)
            o2 = np.argsort(ldst, kind="stable")
            m[s] = dict(degs=degs, node_of_rank=order, rank_of=rank_of,
                        lsrc=lsrc_row[o2], gsrc=gsrc[o2],
                        starts=np.searchsorted(ldst[o2], np.arange(SH)))
        meta.append(m)

    R = [max(1, max(int(meta[c][s]["degs"].max()) for c in range(CORES))) for s in (0, 1)]
    COLS = [[], []]
    for s in (0, 1):
        for j in range(R[s]):
            cj = max(int((meta[c][s]["degs"] > j).sum()) for c in range(CORES))
            COLS[s].append(KCH if j == 0 else max(1, -(-cj // 128)))

    # layer-2 table row of global node n (canonical = system-0 rank order)
    t2row = np.empty(N_NODES, np.int64)
    for c in range(CORES):
        rk = meta[c][0]["rank_of"]
        t2row[c * SH_REAL:(c + 1) * SH_REAL] = c * SH + _srow(rk[:SH_REAL])

    ZROW = _srow(np.int64(SH_REAL + 2))  # a zero pad row within each half's first core

    core_inputs = []
    for c in range(CORES):
        m = meta[c]
        xT = np.zeros((D, SH), np.float32)
        xT[:, :SH_REAL] = x[c * SH_REAL:(c + 1) * SH_REAL].T
        dv = np.zeros(SH, np.float32)
        dv[:SH_REAL] = dinv[c * SH_REAL:(c + 1) * SH_REAL]
        dinv1 = np.ascontiguousarray(dv.reshape(KCH, 128).T)

        nor0 = m[0]["node_of_rank"]
        valid0 = nor0 < SH_REAL
        gnode0 = c * SH_REAL + np.minimum(nor0, SH_REAL - 1)
        dvR = np.where(valid0, dinv[gnode0], 0.0).astype(np.float32)
        dinvR = np.ascontiguousarray(dvR.reshape(KCH, 128).T)
        br = np.where(valid0, np.asarray(batch)[gnode0].astype(np.float64), PADG)
        batchR = np.ascontiguousarray(br.reshape(KCH, 128).T).astype(np.float32)

        idx_l = {1: [[], []], 2: [[], []]}
        for s in (0, 1):
            ms = m[s]
            base = HALF * s
            nedge = len(ms["lsrc"])
            for j in range(R[s]):
                nslots = COLS[s][j] * 128
                ranks = np.arange(nslots)
                live = ranks < int((ms["degs"] > j).sum())
                ln = ms["node_of_rank"][np.minimum(ranks, SH - 1)]
                epos = np.minimum(ms["starts"][ln] + j, max(nedge - 1, 0))
                if nedge:
                    r1 = np.where(live, ms["lsrc"][epos] - base, ZROW)
                    r2 = np.where(live, t2row[ms["gsrc"][epos]] - base, ZROW)
                else:
                    r1 = np.full(nslots, ZROW, np.int64)
                    r2 = np.full(nslots, ZROW, np.int64)
                idx_l[1][s].append(r1)
                idx_l[2][s].append(r2)

        perm01 = _srow(m[1]["rank_of"][nor0])

        core_inputs.append(dict(
            xT=xT, dinv1=dinv1, dinvR=dinvR, batchR=batchR,
            idx1_0=_wrap_idx(np.concatenate(idx_l[1][0])),
            idx1_1=_wrap_idx(np.concatenate(idx_l[1][1])),
            idx2_0=_wrap_idx(np.concatenate(idx_l[2][0])),
            idx2_1=_wrap_idx(np.concatenate(idx_l[2][1])),
            perm=_wrap_idx(perm01),
        ))

    cnts = np.bincount(np.asarray(batch, np.int64), minlength=N_GRAPHS).astype(np.float32)
    return core_inputs, R, COLS, cnts


def _build(R, COLS):
    import concourse.mybir as mybir
    import concourse.tile as tile
    from concourse import bacc, library_config

    f32 = mybir.dt.float32
    bf16 = mybir.dt.bfloat16
    i16 = mybir.dt.int16
    Alu = mybir.AluOpType
    W0_, W1_ = sum(COLS[0]) * 8, sum(COLS[1]) * 8   # wrapped idx widths per layer

    nc = bacc.Bacc(None, target_bir_lowering=False, num_devices=CORES)
    with tile.TileContext(nc) as tc:
        with tc.tile_pool(name="dram", bufs=1, space="DRAM") as dram, \
             tc.tile_pool(name="cst", bufs=1) as cst, \
             tc.tile_pool(name="big", bufs=1) as big, \
             tc.tile_pool(name="tmp", bufs=2) as tmpp, \
             tc.tile_pool(name="stg", bufs=3) as stg, \
             tc.tile_pool(name="pg", bufs=2, space="PSUM") as pg, \
             tc.tile_pool(name="pt", bufs=2, space="PSUM") as pt, \
             tc.tile_pool(name="pp", bufs=1, space="PSUM") as pp:

            # ---------- I/O ----------
            xT_d = dram.tile((D, SH), f32, kind="ExternalInput")
            W1_d = dram.tile((D, D), f32, kind="ExternalInput")
            W2_d = dram.tile((D, D), f32, kind="ExternalInput")
            b1_d = dram.tile((D, 1), f32, kind="ExternalInput")
            b2b_d = dram.tile((D, D), f32, kind="ExternalInput")
            dinv1_d = dram.tile((D, KCH), f32, kind="ExternalInput")
            dinvR_d = dram.tile((D, KCH), f32, kind="ExternalInput")
            batchR_d = dram.tile((D, KCH), f32, kind="ExternalInput")
            iota_d = dram.tile((D, N_GRAPHS), f32, kind="ExternalInput")
            ident_d = dram.tile((D, D), f32, kind="ExternalInput")
            idx_d = {
                (1, 0): dram.tile((128, W0_), i16, kind="ExternalInput", name="idx10"),
                (1, 1): dram.tile((128, W1_), i16, kind="ExternalInput", name="idx11"),
                (2, 0): dram.tile((128, W0_), i16, kind="ExternalInput", name="idx20"),
                (2, 1): dram.tile((128, W1_), i16, kind="ExternalInput", name="idx21"),
            }
            perm_d = dram.tile((128, SH // 16), i16, kind="ExternalInput")
            pool_d = dram.tile((D, N_GRAPHS), f32, kind="ExternalOutput")

            # internal DRAM
            bounce = [dram.tile((128, KCH, D), bf16, name=f"bounce{i}") for i in range(2)]
            table = [dram.tile((TBL, D), bf16, name=f"table{i}") for i in range(2)]
            acc1_d = dram.tile((SH, D), bf16)

            # ---------- SBUF ----------
            nc.gpsimd.load_library(library_config.mlp)
            xT = big.tile([D, SH], f32)
            h1T = big.tile([D, SH], f32)
            acc0 = big.tile([128, KCH, D], bf16)
            acc1 = big.tile([128, KCH, D], bf16)
            W1s = cst.tile([D, D], f32)
            W2s = cst.tile([D, D], f32)
            b1s = cst.tile([D, 1], f32)
            b2b = cst.tile([D, D], f32)
            dinv1 = cst.tile([D, KCH], f32)
            dinvR = cst.tile([D, KCH], f32)
            batchR = cst.tile([D, KCH], f32)
            iota = cst.tile([D, N_GRAPHS], f32)
            ident = cst.tile([D, D], f32)
            idx_sb = {
                (1, 0): cst.tile([128, W0_], i16, name="idx10_sb"),
                (1, 1): cst.tile([128, W1_], i16, name="idx11_sb"),
                (2, 0): cst.tile([128, W0_], i16, name="idx20_sb"),
                (2, 1): cst.tile([128, W1_], i16, name="idx21_sb"),
            }
            perm_sb = cst.tile([128, SH // 16], i16)

            for sb, d in [(xT, xT_d), (W1s, W1_d), (W2s, W2_d), (b1s, b1_d),
                          (b2b, b2b_d), (dinv1, dinv1_d), (dinvR, dinvR_d),
                          (batchR, batchR_d), (iota, iota_d), (ident, ident_d),
                          (perm_sb, perm_d)] + [(idx_sb[k], idx_d[k]) for k in idx_sb]:
                nc.sync.dma_start(out=sb[:], in_=d[:])

            def gemm_ag(lhsT_sb, Wsb, dv_sb, lnum):
                """49 matmuls + dinv scale + batched DMA to bounce, then AllGather."""
                for k0 in range(0, KCH, KB):
                    stage = stg.tile([128, KB, D], bf16, tag="stage")
                    for k in range(k0, k0 + KB):
                        ps = pg.tile([128, D], f32, space="PSUM", tag="pg")
                        nc.tensor.matmul(ps[:], lhsT=lhsT_sb[:, k * 128:(k + 1) * 128],
                                         rhs=Wsb[:], start=True, stop=True)
                        nc.vector.tensor_scalar(
                            out=stage[:, k - k0, :], in0=ps[:],
                            scalar1=dv_sb[:, k:k + 1], scalar2=None, op0=Alu.mult)
                    nc.scalar.dma_start(out=bounce[lnum][:, k0:k0 + KB, :],
                                        in_=stage[:])
                nc.gpsimd.collective_compute(
                    "AllGather", Alu.bypass,
                    replica_groups=[list(range(CORES))],
                    ins=[bounce[lnum].opt()],
                    outs=[table[lnum].opt()])

            def gather_pieces(dst_tile, dst_c0, src, idxap, icol0, cols, accumulate):
                """issue dma_gather in pieces of <= MAXC columns (128 idx each)."""
                for p0 in range(0, cols, MAXC):
                    cc = min(MAXC, cols - p0)
                    n = cc * 128
                    isl = idxap[:, icol0 + p0 * 8: icol0 + (p0 + cc) * 8]
                    if not accumulate:
                        nc.gpsimd.dma_gather(
                            dst_tile[:, dst_c0 + p0: dst_c0 + p0 + cc, :],
                            src, isl, n, n, D)
                    else:
                        t = tmpp.tile([128, MAXC, D], bf16, tag="tmp")
                        nc.gpsimd.dma_gather(t[:, :cc, :], src, isl, n, n, D)
                        nc.vector.tensor_tensor(
                            out=dst_tile[:, dst_c0 + p0: dst_c0 + p0 + cc, :],
                            in0=dst_tile[:, dst_c0 + p0: dst_c0 + p0 + cc, :],
                            in1=t[:, :cc, :], op=Alu.add)

            def rounds(lnum):
                """degree-round gathers for both half-systems + merge into acc0.

                System 1 runs first so its merge traffic (DMA out + permutation
                gather) overlaps system 0's rounds; the merge-add is chunked in
                KB-column pieces so tail columns (low-degree nodes, finalized
                after the first few rounds) release into the epilogue / next
                GEMM while early columns are still accumulating (Tile subtile
                deps)."""
                halves = (table[lnum][0:HALF, :], table[lnum][HALF:TBL, :])
                for s, accs in ((1, acc1), (0, acc0)):
                    icol = 0
                    for j in range(R[s]):
                        cols = COLS[s][j]
                        gather_pieces(accs, 0, halves[s], idx_sb[(lnum + 1, s)],
                                      icol, cols, accumulate=(j > 0))
                        icol += cols * 8
                    if s == 1:
                        nc.scalar.dma_start(out=acc1_d[:], in_=acc1[:])
                tm = tmpp.tile([128, KCH, D], bf16, tag="tmpm")
                for c0 in range(0, KCH, KB):
                    cc = min(KB, KCH - c0)
                    gather_pieces(tm, c0, acc1_d[:], perm_sb, c0 * 8, cc,
                                  accumulate=False)
                    nc.vector.tensor_tensor(
                        out=acc0[:, c0:c0 + cc, :], in0=acc0[:, c0:c0 + cc, :],
                        in1=tm[:, c0:c0 + cc, :], op=Alu.add)

            # ---------------- layer 1 ----------------
            gemm_ag(xT, W1s, dinv1, 0)
            rounds(0)
            # epilogue: h1T[:, chunk] = relu(transpose(acc0*dinvR) + b1)
            for k in range(KCH):
                st = stg.tile([128, D], f32, tag="epi")
                nc.vector.tensor_scalar(out=st[:], in0=acc0[:, k, :],
                                        scalar1=dinvR[:, k:k + 1], scalar2=None,
                                        op0=Alu.mult)
                pst = pt.tile([128, D], f32, space="PSUM", tag="pt")
                nc.tensor.transpose(pst[:], st[:], ident[:])
                nc.scalar.activation(out=h1T[:, k * 128:(k + 1) * 128], in_=pst[:],
                                     func=mybir.ActivationFunctionType.Relu,
                                     bias=b1s[:, :1], scale=1.0)

            # ---------------- layer 2 ----------------
            gemm_ag(h1T, W2s, dinvR, 1)
            rounds(1)
            pool_ps = pp.tile([128, N_GRAPHS], f32, space="PSUM")
            for k in range(KCH):
                h2 = stg.tile([128, D], f32, tag="h2")
                nc.vector.tensor_scalar(out=h2[:], in0=acc0[:, k, :],
                                        scalar1=dinvR[:, k:k + 1], scalar2=None,
                                        op0=Alu.mult)
                nc.vector.tensor_tensor(out=h2[:], in0=h2[:], in1=b2b[:], op=Alu.add)
                h2r = stg.tile([128, D], f32, tag="h2r")
                nc.scalar.activation(out=h2r[:], in_=h2[:],
                                     func=mybir.ActivationFunctionType.Relu,
                                     bias=0.0, scale=1.0)
                G = stg.tile([128, N_GRAPHS], f32, tag="G")
                nc.vector.tensor_scalar(out=G[:], in0=iota[:],
                                        scalar1=batchR[:, k:k + 1], scalar2=None,
                                        op0=Alu.is_equal)
                nc.tensor.matmul(pool_ps[:], lhsT=h2r[:], rhs=G[:],
                                 start=(k == 0), stop=(k == KCH - 1))
            outsb = stg.tile([128, N_GRAPHS], f32, tag="G")
            nc.vector.tensor_copy(out=outsb[:], in_=pool_ps[:])
            nc.sync.dma_start(out=pool_d[:], in_=outsb[:])

    nc.compile()
    names = dict(
        xT=xT_d.name, W1=W1_d.name, W2=W2_d.name, b1=b1_d.name, b2b=b2b_d.name,
        dinv1=dinv1_d.name, dinvR=dinvR_d.name, batchR=batchR_d.name,
        iota=iota_d.name, ident=ident_d.name, perm=perm_d.name,
        i10=idx_d[(1, 0)].name, i11=idx_d[(1, 1)].name,
        i20=idx_d[(2, 0)].name, i21=idx_d[(2, 1)].name,
        out=pool_d.name)
    return nc, names


_cache = {}
_last_in_maps = None


def kernel(x, edge_index, batch, W1, b1, W2, b2, Wl, bl):
    from concourse.bass_utils import run_bass_kernel_spmd

    x = np.asarray(x, np.float32)
    edge_index = np.asarray(edge_index)
    batch = np.asarray(batch)
    W1 = np.asarray(W1, np.float32); b1 = np.asarray(b1, np.float32)
    W2 = np.asarray(W2, np.float32); b2 = np.asarray(b2, np.float32)
    Wl = np.asarray(Wl, np.float32); bl = np.asarray(bl, np.float32)

    core_inputs, R, COLS, cnts = _host_prep(x, edge_index, batch)

    key = (tuple(R), tuple(map(tuple, COLS)))
    if key not in _cache:
        _cache[key] = _build(R, COLS)
    nc, nm = _cache[key]

    iota_b = np.broadcast_to(np.arange(N_GRAPHS, dtype=np.float32)[None, :],
                             (D, N_GRAPHS)).copy()
    b2b = np.broadcast_to(b2[None, :], (D, D)).copy()
    ident = np.eye(D, dtype=np.float32)
    in_maps = []
    for ci in core_inputs:
        in_maps.append({
            nm["xT"]: ci["xT"], nm["W1"]: W1, nm["W2"]: W2,
            nm["b1"]: b1.reshape(D, 1), nm["b2b"]: b2b,
            nm["dinv1"]: ci["dinv1"], nm["dinvR"]: ci["dinvR"],
            nm["batchR"]: ci["batchR"], nm["iota"]: iota_b, nm["ident"]: ident,
            nm["perm"]: ci["perm"],
            nm["i10"]: ci["idx1_0"], nm["i11"]: ci["idx1_1"],
            nm["i20"]: ci["idx2_0"], nm["i21"]: ci["idx2_1"],
        })
    global _last_in_maps
    _last_in_maps = in_maps
    res = run_bass_kernel_spmd(nc, in_maps, list(range(CORES)))
    total = np.zeros((D, N_GRAPHS), np.float32)
    for r in res.results:
        total += r[nm["out"]]
    pooled = (total / np.maximum(cnts, 1.0)[None, :]).T
    return (pooled @ Wl + bl).astype(np.float32)


# revision 6
# speedup vs baseline: 1.2290x; 1.0135x over previous
"""GCN (2x GCNConv + mean-pool + linear) on 8 Trainium2 cores.

Strategy (all on-device in ONE SPMD dispatch):
  - Nodes block-partitioned across 8 cores (6250/core, padded to 6272=49*128).
  - Per layer: each core computes ys = dinv * (h_shard @ W) for its shard
    (49 PE matmuls), AllGather replicates the 50176-row feature table to
    every core's HBM.
  - Message passing = degree-bucketed dma_gather rounds: round j gathers the
    j-th in-edge's source row for every local dst node (nodes rank-ordered by
    in-degree so each round's active set is a prefix; tail slots gather a
    zero pad row).  Accumulation on DVE.  dma_gather indices are int16, so
    the table is split in two 25088-row halves with an independent
    rank/round system per half; the second system's accumulator is merged in
    via one permutation dma_gather through HBM.
  - Epilogue per 128-node chunk: scale by dinv[dst], +bias, ReLU (and a PE
    transpose to feature-major between layer 1 and layer 2).
  - Mean-pool: one-hot graph matrix per chunk (DVE is_equal vs iota) and PE
    matmul accumulation -> per-core [128 feat, 256 graph] partial sums.
  - Host: sum the 8 partials, divide by counts, final [256,128]@[128,10].
"""

import numpy as np

N_NODES = 50000
N_EDGES = 800000
D = 128
N_GRAPHS = 256
CORES = 8
SH_REAL = 6250
KCH = 49
SH = KCH * 128            # 6272
TBL = CORES * SH          # 50176
HALF = 4 * SH             # 25088
PADG = 300.0
KB = 7                    # gemm-out DMA batch
MAXC = 8                  # max gather piece: MAXC*128 indices                    # gemm-out DMA batching (49 = 7*7)


def _srow(i):
    """node/slot index (k*128+p) -> flat row (p*KCH + k) of the (p,k,f) table."""
    return (i % 128) * KCH + i // 128


def _wrap_idx(idx):
    n = len(idx)
    assert n % 16 == 0
    base = np.asarray(idx, np.int16).reshape(-1, 16).T
    return np.ascontiguousarray(np.tile(base, (8, 1)))


def _host_prep(x, edge_index, batch):
    x = np.asarray(x, np.float32)
    src = np.concatenate([edge_index[0], np.arange(N_NODES, dtype=np.int64)]).astype(np.int64)
    dst = np.concatenate([edge_index[1], np.arange(N_NODES, dtype=np.int64)]).astype(np.int64)
    deg = np.bincount(dst, minlength=N_NODES).astype(np.float32)
    dinv = (1.0 / np.sqrt(deg)).astype(np.float32)

    core_of = dst // SH_REAL
    loc_of_dst = dst % SH_REAL
    src_core = src // SH_REAL
    src_loc = src % SH_REAL
    t1row_src = src_core * SH + _srow(src_loc)
    half_of_src = (t1row_src >= HALF).astype(np.int64)

    meta = []
    for c in range(CORES):
        m = {}
        emask = core_of == c
        for s in (0, 1):
            es = emask & (half_of_src == s)
            ldst = loc_of_dst[es]
            lsrc_row = t1row_src[es]
            gsrc = src[es]
            degs = np.bincount(ldst, minlength=SH).astype(np.int64)
            order = np.argsort(-degs, kind="stable")
            rank_of = np.empty(SH, np.int64)
            rank_of[order] = np.arange(SH)
            o2 = np.argsort(ldst, kind="stable")
            m[s] = dict(degs=degs, node_of_rank=order, rank_of=rank_of,
                        lsrc=lsrc_row[o2], gsrc=gsrc[o2],
                        starts=np.searchsorted(ldst[o2], np.arange(SH)))
        meta.append(m)

    R = [max(1, max(int(meta[c][s]["degs"].max()) for c in range(CORES))) for s in (0, 1)]
    COLS = [[], []]
    for s in (0, 1):
        for j in range(R[s]):
            cj = max(int((meta[c][s]["degs"] > j).sum()) for c in range(CORES))
            COLS[s].append(KCH if j == 0 else max(1, -(-cj // 128)))

    # layer-2 table row of global node n (canonical = system-0 rank order)
    t2row = np.empty(N_NODES, np.int64)
    for c in range(CORES):
        rk = meta[c][0]["rank_of"]
        t2row[c * SH_REAL:(c + 1) * SH_REAL] = c * SH + _srow(rk[:SH_REAL])

    ZROW = _srow(np.int64(SH_REAL + 2))  # a zero pad row within each half's first core

    core_inputs = []
    for c in range(CORES):
        m = meta[c]
        xT = np.zeros((D, SH), np.float32)
        xT[:, :SH_REAL] = x[c * SH_REAL:(c + 1) * SH_REAL].T
        dv = np.zeros(SH, np.float32)
        dv[:SH_REAL] = dinv[c * SH_REAL:(c + 1) * SH_REAL]
        dinv1 = np.ascontiguousarray(dv.reshape(KCH, 128).T)

        nor0 = m[0]["node_of_rank"]
        valid0 = nor0 < SH_REAL
        gnode0 = c * SH_REAL + np.minimum(nor0, SH_REAL - 1)
        dvR = np.where(valid0, dinv[gnode0], 0.0).astype(np.float32)
        dinvR = np.ascontiguousarray(dvR.reshape(KCH, 128).T)
        br = np.where(valid0, np.asarray(batch)[gnode0].astype(np.float64), PADG)
        batchR = np.ascontiguousarray(br.reshape(KCH, 128).T).astype(np.float32)

        idx_l = {1: [[], []], 2: [[], []]}
        for s in (0, 1):
            ms = m[s]
            base = HALF * s
            nedge = len(ms["lsrc"])
            for j in range(R[s]):
                nslots = COLS[s][j] * 128
                ranks = np.arange(nslots)
                live = ranks < int((ms["degs"] > j).sum())
                ln = ms["node_of_rank"][np.minimum(ranks, SH - 1)]
                epos = np.minimum(ms["starts"][ln] + j, max(nedge - 1, 0))
                if nedge:
                    r1 = np.where(live, ms["lsrc"][epos] - base, ZROW)
                    r2 = np.where(live, t2row[ms["gsrc"][epos]] - base, ZROW)
                else:
                    r1 = np.full(nslots, ZROW, np.int64)
                    r2 = np.full(nslots, ZROW, np.int64)
                idx_l[1][s].append(r1)
                idx_l[2][s].append(r2)

        perm01 = _srow(m[1]["rank_of"][nor0])

        core_inputs.append(dict(
            xT=xT, dinv1=dinv1, dinvR=dinvR, batchR=batchR,
            idx1_0=_wrap_idx(np.concatenate(idx_l[1][0])),
            idx1_1=_wrap_idx(np.concatenate(idx_l[1][1])),
            idx2_0=_wrap_idx(np.concatenate(idx_l[2][0])),
            idx2_1=_wrap_idx(np.concatenate(idx_l[2][1])),
            perm=_wrap_idx(perm01),
        ))

    cnts = np.bincount(np.asarray(batch, np.int64), minlength=N_GRAPHS).astype(np.float32)
    return core_inputs, R, COLS, cnts


def _build(R, COLS):
    import concourse.mybir as mybir
    import concourse.tile as tile
    from concourse import bacc, library_config

    f32 = mybir.dt.float32
    bf16 = mybir.dt.bfloat16
    i16 = mybir.dt.int16
    Alu = mybir.AluOpType
    W0_, W1_ = sum(COLS[0]) * 8, sum(COLS[1]) * 8   # wrapped idx widths per layer

    nc = bacc.Bacc(None, target_bir_lowering=False, num_devices=CORES)
    with tile.TileContext(nc) as tc:
        with tc.tile_pool(name="dram", bufs=1, space="DRAM") as dram, \
             tc.tile_pool(name="cst", bufs=1) as cst, \
             tc.tile_pool(name="big", bufs=1) as big, \
             tc.tile_pool(name="tmp", bufs=2) as tmpp, \
             tc.tile_pool(name="stg", bufs=3) as stg, \
             tc.tile_pool(name="pg", bufs=2, space="PSUM") as pg, \
             tc.tile_pool(name="pt", bufs=2, space="PSUM") as pt, \
             tc.tile_pool(name="pp", bufs=1, space="PSUM") as pp:

            # ---------- I/O ----------
            xT_d = dram.tile((D, SH), bf16, kind="ExternalInput")
            W1_d = dram.tile((D, D), bf16, kind="ExternalInput")
            W2_d = dram.tile((D, D), bf16, kind="ExternalInput")
            b1_d = dram.tile((D, 1), f32, kind="ExternalInput")
            b2b_d = dram.tile((D, D), f32, kind="ExternalInput")
            dinv1_d = dram.tile((D, KCH), f32, kind="ExternalInput")
            dinvR_d = dram.tile((D, KCH), f32, kind="ExternalInput")
            batchR_d = dram.tile((D, KCH), f32, kind="ExternalInput")
            iota_d = dram.tile((D, N_GRAPHS), f32, kind="ExternalInput")
            ident_d = dram.tile((D, D), f32, kind="ExternalInput")
            idx_d = {
                (1, 0): dram.tile((128, W0_), i16, kind="ExternalInput", name="idx10"),
                (1, 1): dram.tile((128, W1_), i16, kind="ExternalInput", name="idx11"),
                (2, 0): dram.tile((128, W0_), i16, kind="ExternalInput", name="idx20"),
                (2, 1): dram.tile((128, W1_), i16, kind="ExternalInput", name="idx21"),
            }
            perm_d = dram.tile((128, SH // 16), i16, kind="ExternalInput")
            pool_d = dram.tile((D, N_GRAPHS), f32, kind="ExternalOutput")

            # internal DRAM
            bounce = [dram.tile((128, KCH, D), bf16, name=f"bounce{i}") for i in range(2)]
            table = [dram.tile((TBL, D), bf16, name=f"table{i}") for i in range(2)]
            acc1_d = dram.tile((SH, D), bf16)

            # ---------- SBUF ----------
            nc.gpsimd.load_library(library_config.mlp)
            xT = big.tile([D, SH], bf16)
            h1T = big.tile([D, SH], bf16)
            acc0 = big.tile([128, KCH, D], bf16)
            acc1 = big.tile([128, KCH, D], bf16)
            W1s = cst.tile([D, D], bf16)
            W2s = cst.tile([D, D], bf16)
            b1s = cst.tile([D, 1], f32)
            b2b = cst.tile([D, D], f32)
            dinv1 = cst.tile([D, KCH], f32)
            dinvR = cst.tile([D, KCH], f32)
            batchR = cst.tile([D, KCH], f32)
            iota = cst.tile([D, N_GRAPHS], f32)
            ident = cst.tile([D, D], f32)
            idx_sb = {
                (1, 0): cst.tile([128, W0_], i16, name="idx10_sb"),
                (1, 1): cst.tile([128, W1_], i16, name="idx11_sb"),
                (2, 0): cst.tile([128, W0_], i16, name="idx20_sb"),
                (2, 1): cst.tile([128, W1_], i16, name="idx21_sb"),
            }
            perm_sb = cst.tile([128, SH // 16], i16)

            for sb, d in [(xT, xT_d), (W1s, W1_d), (W2s, W2_d), (b1s, b1_d),
                          (b2b, b2b_d), (dinv1, dinv1_d), (dinvR, dinvR_d),
                          (batchR, batchR_d), (iota, iota_d), (ident, ident_d),
                          (perm_sb, perm_d)] + [(idx_sb[k], idx_d[k]) for k in idx_sb]:
                nc.sync.dma_start(out=sb[:], in_=d[:])

            def gemm_ag(lhsT_sb, Wsb, dv_sb, lnum):
                """49 matmuls + dinv scale + batched DMA to bounce, then AllGather."""
                for k0 in range(0, KCH, KB):
                    stage = stg.tile([128, KB, D], bf16, tag="stage")
                    for k in range(k0, k0 + KB):
                        ps = pg.tile([128, D], f32, space="PSUM", tag="pg")
                        nc.tensor.matmul(ps[:], lhsT=lhsT_sb[:, k * 128:(k + 1) * 128],
                                         rhs=Wsb[:], start=True, stop=True)
                        nc.vector.tensor_scalar(
                            out=stage[:, k - k0, :], in0=ps[:],
                            scalar1=dv_sb[:, k:k + 1], scalar2=None, op0=Alu.mult)
                    nc.scalar.dma_start(out=bounce[lnum][:, k0:k0 + KB, :],
                                        in_=stage[:])
                nc.gpsimd.collective_compute(
                    "AllGather", Alu.bypass,
                    replica_groups=[list(range(CORES))],
                    ins=[bounce[lnum].opt()],
                    outs=[table[lnum].opt()])

            def gather_pieces(dst_tile, dst_c0, src, idxap, icol0, cols, accumulate):
                """issue dma_gather in pieces of <= MAXC columns (128 idx each)."""
                for p0 in range(0, cols, MAXC):
                    cc = min(MAXC, cols - p0)
                    n = cc * 128
                    isl = idxap[:, icol0 + p0 * 8: icol0 + (p0 + cc) * 8]
                    if not accumulate:
                        nc.gpsimd.dma_gather(
                            dst_tile[:, dst_c0 + p0: dst_c0 + p0 + cc, :],
                            src, isl, n, n, D)
                    else:
                        t = tmpp.tile([128, MAXC, D], bf16, tag="tmp")
                        nc.gpsimd.dma_gather(t[:, :cc, :], src, isl, n, n, D)
                        nc.vector.tensor_tensor(
                            out=dst_tile[:, dst_c0 + p0: dst_c0 + p0 + cc, :],
                            in0=dst_tile[:, dst_c0 + p0: dst_c0 + p0 + cc, :],
                            in1=t[:, :cc, :], op=Alu.add)

            def rounds(lnum):
                """degree-round gathers for both half-systems + merge into acc0.

                System 1 runs first so its merge traffic (DMA out + permutation
                gather) overlaps system 0's rounds; the merge-add is chunked in
                KB-column pieces so tail columns (low-degree nodes, finalized
                after the first few rounds) release into the epilogue / next
                GEMM while early columns are still accumulating (Tile subtile
                deps)."""
                halves = (table[lnum][0:HALF, :], table[lnum][HALF:TBL, :])
                for s, accs in ((1, acc1), (0, acc0)):
                    icol = 0
                    for j in range(R[s]):
                        cols = COLS[s][j]
                        gather_pieces(accs, 0, halves[s], idx_sb[(lnum + 1, s)],
                                      icol, cols, accumulate=(j > 0))
                        icol += cols * 8
                    if s == 1:
                        nc.scalar.dma_start(out=acc1_d[:], in_=acc1[:])
                tm = tmpp.tile([128, KCH, D], bf16, tag="tmpm")
                for c0 in range(0, KCH, KB):
                    cc = min(KB, KCH - c0)
                    gather_pieces(tm, c0, acc1_d[:], perm_sb, c0 * 8, cc,
                                  accumulate=False)
                    nc.vector.tensor_tensor(
                        out=acc0[:, c0:c0 + cc, :], in0=acc0[:, c0:c0 + cc, :],
                        in1=tm[:, c0:c0 + cc, :], op=Alu.add)

            # ---------------- layer 1 ----------------
            gemm_ag(xT, W1s, dinv1, 0)
            rounds(0)
            # epilogue: h1T[:, chunk] = relu(transpose(acc0*dinvR) + b1)
            for k in range(KCH):
                st = stg.tile([128, D], f32, tag="epi")
                nc.vector.tensor_scalar(out=st[:], in0=acc0[:, k, :],
                                        scalar1=dinvR[:, k:k + 1], scalar2=None,
                                        op0=Alu.mult)
                pst = pt.tile([128, D], f32, space="PSUM", tag="pt")
                nc.tensor.transpose(pst[:], st[:], ident[:])
                nc.scalar.activation(out=h1T[:, k * 128:(k + 1) * 128], in_=pst[:],
                                     func=mybir.ActivationFunctionType.Relu,
                                     bias=b1s[:, :1], scale=1.0)

            # ---------------- layer 2 ----------------
            gemm_ag(h1T, W2s, dinvR, 1)
            rounds(1)
            pool_ps = pp.tile([128, N_GRAPHS], f32, space="PSUM")
            for k in range(KCH):
                h2 = stg.tile([128, D], f32, tag="h2")
                nc.vector.tensor_scalar(out=h2[:], in0=acc0[:, k, :],
                                        scalar1=dinvR[:, k:k + 1], scalar2=None,
                                        op0=Alu.mult)
                nc.vector.tensor_tensor(out=h2[:], in0=h2[:], in1=b2b[:], op=Alu.add)
                h2r = stg.tile([128, D], f32, tag="h2r")
                nc.scalar.activation(out=h2r[:], in_=h2[:],
                                     func=mybir.ActivationFunctionType.Relu,
                                     bias=0.0, scale=1.0)
                G = stg.tile([128, N_GRAPHS], f32, tag="G")
                nc.vector.tensor_scalar(out=G[:], in0=iota[:],
                                        scalar1=batchR[:, k:k + 1], scalar2=None,
                                        op0=Alu.is_equal)
                nc.tensor.matmul(pool_ps[:], lhsT=h2r[:], rhs=G[:],
                                 start=(k == 0), stop=(k == KCH - 1))
            outsb = stg.tile([128, N_GRAPHS], f32, tag="G")
            nc.vector.tensor_copy(out=outsb[:], in_=pool_ps[:])
            nc.sync.dma_start(out=pool_d[:], in_=outsb[:])

    nc.compile()
    names = dict(
        xT=xT_d.name, W1=W1_d.name, W2=W2_d.name, b1=b1_d.name, b2b=b2b_d.name,
        dinv1=dinv1_d.name, dinvR=dinvR_d.name, batchR=batchR_d.name,
        iota=iota_d.name, ident=ident_d.name, perm=perm_d.name,
        i10=idx_d[(1, 0)].name, i11=idx_d[(1, 1)].name,
        i20=idx_d[(2, 0)].name, i21=idx_d[(2, 1)].name,
        out=pool_d.name)
    return nc, names


_cache = {}
_last_in_maps = None


def kernel(x, edge_index, batch, W1, b1, W2, b2, Wl, bl):
    from concourse.bass_utils import run_bass_kernel_spmd

    x = np.asarray(x, np.float32)
    edge_index = np.asarray(edge_index)
    batch = np.asarray(batch)
    W1 = np.asarray(W1, np.float32); b1 = np.asarray(b1, np.float32)
    W2 = np.asarray(W2, np.float32); b2 = np.asarray(b2, np.float32)
    Wl = np.asarray(Wl, np.float32); bl = np.asarray(bl, np.float32)

    core_inputs, R, COLS, cnts = _host_prep(x, edge_index, batch)

    key = (tuple(R), tuple(map(tuple, COLS)))
    if key not in _cache:
        _cache[key] = _build(R, COLS)
    nc, nm = _cache[key]

    import ml_dtypes
    bf = ml_dtypes.bfloat16
    iota_b = np.broadcast_to(np.arange(N_GRAPHS, dtype=np.float32)[None, :],
                             (D, N_GRAPHS)).copy()
    b2b = np.broadcast_to(b2[None, :], (D, D)).copy()
    ident = np.eye(D, dtype=np.float32)
    in_maps = []
    for ci in core_inputs:
        in_maps.append({
            nm["xT"]: ci["xT"].astype(bf), nm["W1"]: W1.astype(bf), nm["W2"]: W2.astype(bf),
            nm["b1"]: b1.reshape(D, 1), nm["b2b"]: b2b,
            nm["dinv1"]: ci["dinv1"], nm["dinvR"]: ci["dinvR"],
            nm["batchR"]: ci["batchR"], nm["iota"]: iota_b, nm["ident"]: ident,
            nm["perm"]: ci["perm"],
            nm["i10"]: ci["idx1_0"], nm["i11"]: ci["idx1_1"],
            nm["i20"]: ci["idx2_0"], nm["i21"]: ci["idx2_1"],
        })
    global _last_in_maps
    _last_in_maps = in_maps
    res = run_bass_kernel_spmd(nc, in_maps, list(range(CORES)))
    total = np.zeros((D, N_GRAPHS), np.float32)
    for r in res.results:
        total += r[nm["out"]]
    pooled = (total / np.maximum(cnts, 1.0)[None, :]).T
    return (pooled @ Wl + bl).astype(np.float32)
